# revision 1
# baseline (speedup 1.0000x reference)
"""MultiHeadLatentAttention TRN2 kernel.

Sharding: 8 cores = 2 batches x 4 head-groups (4 heads of 128 dims each).
Each core computes, for its (batch, 4 heads):
    qT_h = Wq_h^T xT          [hd, S]     (fp16 matmuls, fp32 psum)
    latT = Wdkv^T xT          [256, S]
    kT_h = Wuk_h^T latT       [hd, S]
    v_h  = latT^T Wuv_h       [S, hd]
    scoresT = k qT            [keys, q]   (transposed scores: no transposes needed)
    expT = exp(scale*scoresT) (causal: skip invalid blocks, tri-mask diagonal)
    den  = ones^T expT        [128, q]    (all-ones stationary matmul = sum over
                                           keys AND broadcast across partitions)
    ctxT = v^T expT / den     [hd, q]
    part = sum_h ctxT_h^T Wout_h  [S, dout]  (row-parallel out-proj partial)
Host sums the 4 partials per batch and adds b_out.
"""

import sys

_BASS_REPO = "/opt/trn_rl_repo"
if _BASS_REPO not in sys.path:
    sys.path.insert(0, _BASS_REPO)

import numpy as np

import concourse.bass as bass  # noqa: F401  (import keeps bass registered)
import concourse.mybir as mybir
import concourse.tile as tile
from concourse import bacc, bass_utils

F32 = mybir.dt.float32
F16 = mybir.dt.float16

B = 2
S = 2048
DIN = 2048
DOUT = 2048
NH = 16
HD = 128
LAT = 256
NCORES = 8
HEADS_PER_CORE = 4
COLS_PER_CORE = HEADS_PER_CORE * HD  # 512

KC = DIN // 128  # 16 contraction chunks over d_in
NB = S // 512    # 4 blocks of 512 over S
NT = S // 128    # 16 tiles of 128 over S
SCALE = 1.0 / float(np.sqrt(HD))

_CACHE = {}


def _build():
    nc = bacc.Bacc("TRN2", target_bir_lowering=False, debug=False,
                   num_devices=NCORES)

    xt_d = nc.dram_tensor("xt", [DIN, S], F16, kind="ExternalInput")
    wq_d = nc.dram_tensor("wq", [DIN, COLS_PER_CORE], F16, kind="ExternalInput")
    wdkv_d = nc.dram_tensor("wdkv", [DIN, LAT], F16, kind="ExternalInput")
    wuk_d = nc.dram_tensor("wuk", [LAT, COLS_PER_CORE], F16, kind="ExternalInput")
    wuv_d = nc.dram_tensor("wuv", [LAT, COLS_PER_CORE], F16, kind="ExternalInput")
    wout_d = nc.dram_tensor("wout", [COLS_PER_CORE, DOUT], F16, kind="ExternalInput")
    mask_d = nc.dram_tensor("mask", [128, 128], F16, kind="ExternalInput")
    out_d = nc.dram_tensor("out", [S, DOUT], F32, kind="ExternalOutput")

    Exp = mybir.ActivationFunctionType.Exp

    with tile.TileContext(nc) as tc:
        with (
            tc.tile_pool(name="consts", bufs=1) as cpool,
            tc.tile_pool(name="wts", bufs=1) as wpool,
            tc.tile_pool(name="acts", bufs=1) as apool,
            tc.tile_pool(name="temps", bufs=1) as tpool,
            tc.tile_pool(name="pproj", bufs=2, space="PSUM") as pproj,
        ):
            # ---- constants ----
            ones_t = cpool.tile([128, 128], F16, name="ones_t", tag="ones_t")
            nc.vector.memset(ones_t[:], 1.0)
            mask_t = cpool.tile([128, 128], F16, name="mask_t", tag="mask_t")
            nc.sync.dma_start(mask_t[:], mask_d.ap())

            # ---- weights ----
            wq = []
            wdkv = []
            for k in range(KC):
                t = wpool.tile([128, COLS_PER_CORE], F16, name=f"wq{k}", tag=f"wq{k}")
                nc.sync.dma_start(t[:], wq_d.ap()[128 * k:128 * (k + 1), :])
                wq.append(t)
                t = wpool.tile([128, LAT], F16, name=f"wdkv{k}", tag=f"wdkv{k}")
                nc.sync.dma_start(t[:], wdkv_d.ap()[128 * k:128 * (k + 1), :])
                wdkv.append(t)
            wuk = []
            wuv = []
            for m in range(LAT // 128):
                t = wpool.tile([128, COLS_PER_CORE], F16, name=f"wuk{m}", tag=f"wuk{m}")
                nc.sync.dma_start(t[:], wuk_d.ap()[128 * m:128 * (m + 1), :])
                wuk.append(t)
                t = wpool.tile([128, COLS_PER_CORE], F16, name=f"wuv{m}", tag=f"wuv{m}")
                nc.sync.dma_start(t[:], wuv_d.ap()[128 * m:128 * (m + 1), :])
                wuv.append(t)
            wout = []
            for h in range(HEADS_PER_CORE):
                t = wpool.tile([128, DOUT], F16, name=f"wout{h}", tag=f"wout{h}")
                nc.sync.dma_start(t[:], wout_d.ap()[128 * h:128 * (h + 1), :])
                wout.append(t)

            # ---- persistent activations ----
            latT = [apool.tile([128, S], F16, name=f"latT{m}", tag=f"latT{m}")
                    for m in range(LAT // 128)]
            qT = [apool.tile([128, S], F16, name=f"qT{h}", tag=f"qT{h}")
                  for h in range(HEADS_PER_CORE)]
            kT = [apool.tile([128, S], F16, name=f"kT{h}", tag=f"kT{h}")
                  for h in range(HEADS_PER_CORE)]
            vt = [apool.tile([128, S], F16, name=f"vt{h}", tag=f"vt{h}")
                  for h in range(HEADS_PER_CORE)]
            ctxT = [apool.tile([128, S], F16, name=f"ctxT{h}", tag=f"ctxT{h}")
                    for h in range(HEADS_PER_CORE)]

            with tc.tile_pool(name="xts", bufs=1) as xpool:
                xt = []
                for k in range(KC):
                    t = xpool.tile([128, S], F16, name=f"xt{k}", tag=f"xt{k}")
                    nc.sync.dma_start(t[:], xt_d.ap()[128 * k:128 * (k + 1), :])
                    xt.append(t)

                # ---- latent projection: latT = Wdkv^T xT ----
                for m in range(LAT // 128):
                    for sb in range(NB):
                        pl = pproj.tile([128, 512], F32, name="pl", tag="pp")
                        for k in range(KC):
                            nc.tensor.matmul(
                                pl[:], wdkv[k][:, 128 * m:128 * (m + 1)],
                                xt[k][:, 512 * sb:512 * (sb + 1)],
                                start=(k == 0), stop=(k == KC - 1))
                        nc.scalar.copy(latT[m][:, 512 * sb:512 * (sb + 1)], pl[:])

                for h in range(HEADS_PER_CORE):
                    hs = slice(128 * h, 128 * (h + 1))
                    # ---- qT_h = Wq_h^T xT ----
                    for sb in range(NB):
                        pq = pproj.tile([128, 512], F32, name="pq", tag="pp")
                        for k in range(KC):
                            nc.tensor.matmul(
                                pq[:], wq[k][:, hs],
                                xt[k][:, 512 * sb:512 * (sb + 1)],
                                start=(k == 0), stop=(k == KC - 1))
                        nc.scalar.copy(qT[h][:, 512 * sb:512 * (sb + 1)], pq[:])
                    # ---- kT_h = Wuk_h^T latT ----
                    for sb in range(NB):
                        pk = pproj.tile([128, 512], F32, name="pk", tag="pp")
                        for m in range(LAT // 128):
                            nc.tensor.matmul(
                                pk[:], wuk[m][:, hs],
                                latT[m][:, 512 * sb:512 * (sb + 1)],
                                start=(m == 0), stop=(m == LAT // 128 - 1))
                        nc.scalar.copy(kT[h][:, 512 * sb:512 * (sb + 1)], pk[:])
                    # ---- v_h = latT^T Wuv_h, stored [128, st*128+d] ----
                    for st4 in range(NB):
                        pv = pproj.tile([128, 512], F32, name="pv", tag="pp")
                        for j in range(4):
                            stt = 4 * st4 + j
                            for m in range(LAT // 128):
                                nc.tensor.matmul(
                                    pv[:, 128 * j:128 * (j + 1)],
                                    latT[m][:, 128 * stt:128 * (stt + 1)],
                                    wuv[m][:, hs],
                                    start=(m == 0), stop=(m == LAT // 128 - 1))
                        nc.scalar.copy(vt[h][:, 512 * st4:512 * (st4 + 1)], pv[:])

            # ---- attention (per head, transposed scores) ----
            with (
                tc.tile_pool(name="psc", bufs=2, space="PSUM") as psc,
                tc.tile_pool(name="pctx", bufs=2, space="PSUM") as pctx,
                tc.tile_pool(name="pden", bufs=2, space="PSUM") as pden,
            ):
                for h in range(HEADS_PER_CORE):
                    for qb in range(NB):
                        ps_ctx = pctx.tile([128, 512], F32, name="ps_ctx", tag="ctx")
                        ps_den = pden.tile([128, 512], F32, name="ps_den", tag="den")
                        nkt = 4 * qb + 4
                        for kt in range(nkt):
                            dj = kt - 4 * qb
                            col0 = 128 * dj if dj >= 0 else 0
                            qlo = 512 * qb + col0
                            qhi = 512 * (qb + 1)
                            ps_sc = psc.tile([128, 512], F32, name="ps_sc", tag="sc")
                            nc.tensor.matmul(
                                ps_sc[:, col0:512],
                                kT[h][:, 128 * kt:128 * (kt + 1)],
                                qT[h][:, qlo:qhi],
                                start=True, stop=True)
                            ex = tpool.tile([128, 512], F16, name="ex", tag="ex",
                                            bufs=3)
                            nc.scalar.activation(ex[:, col0:512], ps_sc[:, col0:512],
                                                 Exp, scale=SCALE)
                            if dj >= 0:
                                nc.vector.tensor_mul(ex[:, col0:col0 + 128],
                                                     ex[:, col0:col0 + 128],
                                                     mask_t[:])
                            nc.tensor.matmul(
                                ps_ctx[:, col0:512],
                                vt[h][:, 128 * kt:128 * (kt + 1)],
                                ex[:, col0:512],
                                start=(kt == 0), stop=(kt == nkt - 1))
                            nc.tensor.matmul(
                                ps_den[:, col0:512],
                                ones_t[:],
                                ex[:, col0:512],
                                start=(kt == 0), stop=(kt == nkt - 1))
                        rden = tpool.tile([128, 512], F32, name="rden", tag="rden",
                                          bufs=2)
                        nc.vector.reciprocal(rden[:], ps_den[:])
                        nc.vector.tensor_mul(ctxT[h][:, 512 * qb:512 * (qb + 1)],
                                             ps_ctx[:], rden[:])

            # ---- out projection: part = sum_h ctxT_h^T Wout_h ----
            with tc.tile_pool(name="pout", bufs=2, space="PSUM") as pout:
                for stt in range(NT):
                    for ob in range(NB):
                        po = pout.tile([128, 512], F32, name="po", tag="po")
                        for h in range(HEADS_PER_CORE):
                            nc.tensor.matmul(
                                po[:],
                                ctxT[h][:, 128 * stt:128 * (stt + 1)],
                                wout[h][:, 512 * ob:512 * (ob + 1)],
                                start=(h == 0), stop=(h == HEADS_PER_CORE - 1))
                        osb = tpool.tile([128, 512], F32, name="osb", tag="osb",
                                         bufs=3)
                        nc.scalar.copy(osb[:], po[:])
                        nc.sync.dma_start(
                            out_d.ap()[128 * stt:128 * (stt + 1),
                                       512 * ob:512 * (ob + 1)],
                            osb[:])

    nc.compile()
    return nc


def _get_nc():
    if "nc" not in _CACHE:
        _CACHE["nc"] = _build()
    return _CACHE["nc"]


def _make_in_maps(x, W_query, W_DKV, W_UK, W_UV, W_out):
    mask = np.triu(np.ones((128, 128), dtype=np.float16))
    wdkv16 = W_DKV.astype(np.float16)
    xT16 = [x[b].T.astype(np.float16) for b in range(B)]
    in_maps = []
    for c in range(NCORES):
        b = c // 4
        g = c % 4
        cols = slice(512 * g, 512 * (g + 1))
        in_maps.append({
            "xt": xT16[b],
            "wq": W_query[:, cols].astype(np.float16),
            "wdkv": wdkv16,
            "wuk": W_UK[:, cols].astype(np.float16),
            "wuv": W_UV[:, cols].astype(np.float16),
            "wout": W_out[cols, :].astype(np.float16),
            "mask": mask,
        })
    return in_maps


def run_on_device(x, W_query, W_DKV, W_UK, W_UV, W_out, **run_kwargs):
    nc = _get_nc()
    in_maps = _make_in_maps(x, W_query, W_DKV, W_UK, W_UV, W_out)
    return bass_utils.run_bass_kernel_spmd(
        nc, in_maps, core_ids=list(range(NCORES)), **run_kwargs)


def kernel(x, W_query, W_DKV, W_UK, W_UV, W_out, b_out):
    x = np.asarray(x, dtype=np.float32)
    W_query = np.asarray(W_query, dtype=np.float32)
    W_DKV = np.asarray(W_DKV, dtype=np.float32)
    W_UK = np.asarray(W_UK, dtype=np.float32)
    W_UV = np.asarray(W_UV, dtype=np.float32)
    W_out = np.asarray(W_out, dtype=np.float32)
    b_out = np.asarray(b_out, dtype=np.float32)

    res = run_on_device(x, W_query, W_DKV, W_UK, W_UV, W_out)
    out = np.empty((B, S, DOUT), dtype=np.float32)
    for b in range(B):
        acc = res.results[4 * b]["out"].copy()
        for g in range(1, 4):
            acc += res.results[4 * b + g]["out"]
        out[b] = acc + b_out[None, :]
    return out


# revision 34
# speedup vs baseline: 20997.3810x; 20997.3810x over previous
"""MultiHeadLatentAttention TRN2 kernel.

Sharding: 8 cores = 2 batches x 4 head-groups (4 heads of 128 dims each).
Each core computes, for its (batch, 4 heads):
    qT_h = Wq_h^T xT          [hd, S]     (fp16 matmuls, fp32 psum)
    latT = Wdkv^T xT          [256, S]
    kT_h = Wuk_h^T latT       [hd, S]
    v_h  = latT^T Wuv_h       [S, hd]
    scoresT = k qT            [keys, q]   (transposed scores: no transposes needed)
    expT = exp(scale*scoresT) (causal: skip invalid blocks, tri-mask diagonal)
    den  = ones^T expT        [128, q]    (all-ones stationary matmul = sum over
                                           keys AND broadcast across partitions)
    ctxT = v^T expT / den     [hd, q]
    part = sum_h ctxT_h^T Wout_h  [S, dout]  (row-parallel out-proj partial)
Host sums the 4 partials per batch and adds b_out.
"""

import sys

_BASS_REPO = "/opt/trn_rl_repo"
if _BASS_REPO not in sys.path:
    sys.path.insert(0, _BASS_REPO)

import numpy as np

import concourse.bass as bass  # noqa: F401
import concourse.mybir as mybir
import concourse.tile as tile
from concourse import bacc, bass_utils

F32 = mybir.dt.float32
F16 = mybir.dt.float16

B = 2
S = 2048
DIN = 2048
DOUT = 2048
NH = 16
HD = 128
LAT = 256
NCORES = 8
HEADS_PER_CORE = 4
COLS_PER_CORE = HEADS_PER_CORE * HD  # 512

KC = DIN // 128  # 16 contraction chunks over d_in
NB = S // 512    # 4 blocks of 512 over S
NT = S // 128    # 16 tiles of 128 over S
SCALE = 1.0 / float(np.sqrt(HD))

_CACHE = {}


def _build():
    nc = bacc.Bacc("TRN2", target_bir_lowering=False, debug=False,
                   num_devices=NCORES)

    xt_d = nc.dram_tensor("xt", [DIN, S], F16, kind="ExternalInput")
    wq_d = nc.dram_tensor("wq", [DIN, COLS_PER_CORE], F16, kind="ExternalInput")
    wdkv_d = nc.dram_tensor("wdkv", [DIN, LAT], F16, kind="ExternalInput")
    wuk_d = nc.dram_tensor("wuk", [LAT, COLS_PER_CORE], F16, kind="ExternalInput")
    wuv_d = nc.dram_tensor("wuv", [LAT, COLS_PER_CORE], F16, kind="ExternalInput")
    wout_d = nc.dram_tensor("wout", [COLS_PER_CORE, DOUT], F16, kind="ExternalInput")
    mask_d = nc.dram_tensor("mask", [128, 128], F16, kind="ExternalInput")
    out_d = nc.dram_tensor("out", [S, DOUT], F32, kind="ExternalOutput")

    Exp = mybir.ActivationFunctionType.Exp

    with tile.TileContext(nc) as tc:
        with (
            tc.tile_pool(name="consts", bufs=1) as cpool,
            tc.tile_pool(name="wts", bufs=1) as wpool,
            tc.tile_pool(name="acts", bufs=1) as apool,
            tc.tile_pool(name="temps", bufs=1) as tpool,
        ):
            # ---- constants ----
            ones_t = cpool.tile([128, 512], F16, name="ones_t", tag="ones_t")
            nc.vector.memset(ones_t[:], 1.0)
            neg_t = cpool.tile([128, 128], F16, name="neg_t", tag="neg_t")
            nc.vector.memset(neg_t[:], -30000.0)
            mask_t = cpool.tile([128, 128], F16, name="mask_t", tag="mask_t")
            nc.scalar.dma_start(mask_t[:], mask_d.ap())

            # ---- weights ----
            # xt/wdkv stream on the sync HWDGE ring (feed the first matmuls);
            # everything else loads in parallel on the scalar ring.
            wq = []
            wdkv = []
            xt = []
            for k in range(KC):
                t = wpool.tile([128, S], F16, name=f"xt{k}", tag=f"xt{k}")
                eng = nc.sync if k % 2 == 0 else nc.scalar
                eng.dma_start(t[:], xt_d.ap()[128 * k:128 * (k + 1), :])
                xt.append(t)
                t = wpool.tile([128, LAT], F16, name=f"wdkv{k}", tag=f"wdkv{k}")
                nc.sync.dma_start(t[:], wdkv_d.ap()[128 * k:128 * (k + 1), :])
                wdkv.append(t)
                t = wpool.tile([128, COLS_PER_CORE], F16, name=f"wq{k}", tag=f"wq{k}")
                nc.scalar.dma_start(t[:], wq_d.ap()[128 * k:128 * (k + 1), :])
                wq.append(t)
            wuk = []
            wuv = []
            for m in range(LAT // 128):
                t = wpool.tile([128, COLS_PER_CORE], F16, name=f"wuk{m}", tag=f"wuk{m}")
                nc.scalar.dma_start(t[:], wuk_d.ap()[128 * m:128 * (m + 1), :])
                wuk.append(t)
                t = wpool.tile([128, COLS_PER_CORE], F16, name=f"wuv{m}", tag=f"wuv{m}")
                nc.scalar.dma_start(t[:], wuv_d.ap()[128 * m:128 * (m + 1), :])
                wuv.append(t)
            wout = []
            for h in range(HEADS_PER_CORE):
                t = wpool.tile([128, DOUT], F16, name=f"wout{h}", tag=f"wout{h}")
                nc.scalar.dma_start(t[:], wout_d.ap()[128 * h:128 * (h + 1), :])
                wout.append(t)

            # ---- persistent activations ----
            latT = [apool.tile([128, S], F16, name=f"latT{m}", tag=f"latT{m}")
                    for m in range(LAT // 128)]
            qT = [apool.tile([128, S], F16, name=f"qT{h}", tag=f"qT{h}")
                  for h in range(HEADS_PER_CORE)]
            kT = [apool.tile([128, S], F16, name=f"kT{h}", tag=f"kT{h}")
                  for h in range(HEADS_PER_CORE)]
            vt = [apool.tile([128, S], F16, name=f"vt{h}", tag=f"vt{h}")
                  for h in range(HEADS_PER_CORE)]
            ctxT = [apool.tile([128, S], F16, name=f"ctxT{h}", tag=f"ctxT{h}")
                    for h in range(HEADS_PER_CORE)]

            # ================= phase 1: projections =================
            with tc.tile_pool(name="pproj", bufs=8, space="PSUM") as pproj:
                # PE warmup: HAM-warm the array while input DMAs stream in.
                warm = pproj.tile([128, 512], F32, name="warm", tag="pp")
                for _ in range(40):
                    nc.tensor.matmul(warm[:, 0:128], ones_t[:, 0:128],
                                     ones_t[:, 0:128], start=True, stop=True)

                def kmajor(groups, lhs_of, rhs_of, nk, out_of, copy_eng="scalar"):
                    """Accumulate len(groups) psum banks over nk chunks,
                    chunk-major so compute starts on the first DMA."""
                    pls = [pproj.tile([128, 512], F32, name=f"pp{i}", tag="pp")
                           for i in range(len(groups))]
                    for k in range(nk):
                        for i, g in enumerate(groups):
                            nc.tensor.matmul(pls[i][:], lhs_of(k, g), rhs_of(k, g),
                                             start=(k == 0), stop=(k == nk - 1))
                    for i, g in enumerate(groups):
                        if copy_eng == "scalar":
                            nc.scalar.copy(out_of(g), pls[i][:])
                        else:
                            nc.vector.tensor_copy(out_of(g), pls[i][:])

                # latT = Wdkv^T xT   (8 groups: 2 m x 4 sb)
                kmajor(
                    [(m, sb) for m in range(2) for sb in range(NB)],
                    lambda k, g: wdkv[k][:, 128 * g[0]:128 * (g[0] + 1)],
                    lambda k, g: xt[k][:, 512 * g[1]:512 * (g[1] + 1)],
                    KC,
                    lambda g: latT[g[0]][:, 512 * g[1]:512 * (g[1] + 1)])

                # qT_h = Wq_h^T xT   (two batches of 8 groups: 2 h x 4 sb)
                for h0 in (0, 2):
                    kmajor(
                        [(h0 + dh, sb) for dh in range(2) for sb in range(NB)],
                        lambda k, g: wq[k][:, 128 * g[0]:128 * (g[0] + 1)],
                        lambda k, g: xt[k][:, 512 * g[1]:512 * (g[1] + 1)],
                        KC,
                        lambda g: qT[g[0]][:, 512 * g[1]:512 * (g[1] + 1)])

                # kT_h = Wuk_h^T latT
                kmajor(
                    [(h, sb) for h in range(2) for sb in range(NB)],
                    lambda k, g: wuk[k][:, 128 * g[0]:128 * (g[0] + 1)],
                    lambda k, g: latT[k][:, 512 * g[1]:512 * (g[1] + 1)],
                    2,
                    lambda g: kT[g[0]][:, 512 * g[1]:512 * (g[1] + 1)],
                    copy_eng="vector")
                kmajor(
                    [(h, sb) for h in (2, 3) for sb in range(NB)],
                    lambda k, g: wuk[k][:, 128 * g[0]:128 * (g[0] + 1)],
                    lambda k, g: latT[k][:, 512 * g[1]:512 * (g[1] + 1)],
                    2,
                    lambda g: kT[g[0]][:, 512 * g[1]:512 * (g[1] + 1)],
                    copy_eng="vector")

                # v_h = latT^T Wuv_h, stored [128, st*128+d]
                for h in range(HEADS_PER_CORE):
                    hs = slice(128 * h, 128 * (h + 1))
                    for st4 in range(NB):
                        pv = pproj.tile([128, 512], F32, name="pv", tag="pp")
                        for j in range(4):
                            stt = 4 * st4 + j
                            for m in range(LAT // 128):
                                nc.tensor.matmul(
                                    pv[:, 128 * j:128 * (j + 1)],
                                    latT[m][:, 128 * stt:128 * (stt + 1)],
                                    wuv[m][:, hs],
                                    start=(m == 0), stop=(m == LAT // 128 - 1))
                        nc.vector.tensor_copy(vt[h][:, 512 * st4:512 * (st4 + 1)],
                                              pv[:])


            # ========= phase 2: attention + interleaved out-proj =========
            # key tiles processed in pairs -> one [128,1024] exp per pair.
            # qb-outer / h-inner so each q-block's out-projection (PE-heavy,
            # ACT-idle) overlaps the next block's ACT-paced attention.
            with (
                tc.tile_pool(name="psc", bufs=2, space="PSUM") as psc,
                tc.tile_pool(name="pctx", bufs=2, space="PSUM") as pctx,
                tc.tile_pool(name="pden", bufs=2, space="PSUM") as pden,
            ):
                for qb in range(NB):
                    for h in range(HEADS_PER_CORE):
                        ps_ctx = pctx.tile([128, 512], F32, name="ps_ctx", tag="ctx")
                        ps_den = pden.tile([128, 512], F32, name="ps_den", tag="den")
                        nkt = 4 * qb + 4
                        for kt0 in range(0, nkt, 2):
                            pair = (kt0, kt0 + 1)
                            # valid q start (block-local) per kt; pair shares
                            # the wider (earlier) start col0 of ktA
                            djA = pair[0] - 4 * qb
                            col0 = 128 * djA if djA >= 0 else 0
                            qhi = 512 * (qb + 1)
                            ps_sc = psc.tile([128, 1024], F32, name="ps_sc",
                                             tag="sc")
                            ex = tpool.tile([128, 1024], F16, name="ex", tag="ex",
                                            bufs=3)
                            for half, kt in enumerate(pair):
                                dj = kt - 4 * qb
                                c = 128 * dj if dj >= 0 else 0
                                if c > 0:
                                    # fill the invalid strip with -huge so the
                                    # wide exp below lands exact zeros there
                                    nc.tensor.matmul(
                                        ps_sc[:, 512 * half:512 * half + c],
                                        neg_t[:], ones_t[:, 0:c],
                                        start=True, stop=True,
                                        skip_group_check=True)
                                nc.tensor.matmul(
                                    ps_sc[:, 512 * half + c:512 * (half + 1)],
                                    kT[h][:, 128 * kt:128 * (kt + 1)],
                                    qT[h][:, 512 * qb + c:qhi],
                                    start=True, stop=True,
                                    skip_group_check=True)
                            # one wide exp for the pair (psum -> sbuf fp16)
                            nc.scalar.activation(ex[:, col0:1024],
                                                 ps_sc[:, col0:1024], Exp,
                                                 scale=SCALE)
                            for half, kt in enumerate(pair):
                                dj = kt - 4 * qb
                                if dj >= 0:
                                    c = 128 * dj
                                    nc.vector.tensor_mul(
                                        ex[:, 512 * half + c:512 * half + c + 128],
                                        ex[:, 512 * half + c:512 * half + c + 128],
                                        mask_t[:])
                            # pair-sum on DVE halves the denominator matmuls
                            exs = tpool.tile([128, 512], F16, name="exs",
                                             tag="exs", bufs=3)
                            nc.vector.tensor_add(exs[:, col0:512],
                                                 ex[:, col0:512],
                                                 ex[:, 512 + col0:1024])
                            nc.tensor.matmul(
                                ps_den[:, col0:512],
                                ones_t[:, 0:128],
                                exs[:, col0:512],
                                start=(kt0 == 0), stop=(kt0 == nkt - 2))
                            for half, kt in enumerate(pair):
                                nc.tensor.matmul(
                                    ps_ctx[:, col0:512],
                                    vt[h][:, 128 * kt:128 * (kt + 1)],
                                    ex[:, 512 * half + col0:512 * (half + 1)],
                                    start=(kt0 == 0 and half == 0),
                                    stop=(kt == nkt - 1))
                        rden = tpool.tile([128, 512], F32, name="rden", tag="rden",
                                          bufs=2)
                        nc.vector.reciprocal_approx_fast(rden[:], ps_den[:])
                        nc.vector.tensor_mul(ctxT[h][:, 512 * qb:512 * (qb + 1)],
                                             ps_ctx[:], rden[:])

                    # out-projection for this q-block's 4 S-tiles
                    # (psum slots shared with the den tag)
                    for stt in range(4 * qb, 4 * qb + 4):
                        for ob in range(NB):
                            po = pden.tile([128, 512], F32, name="po", tag="den")
                            for h in range(HEADS_PER_CORE):
                                nc.tensor.matmul(
                                    po[:],
                                    ctxT[h][:, 128 * stt:128 * (stt + 1)],
                                    wout[h][:, 512 * ob:512 * (ob + 1)],
                                    start=(h == 0), stop=(h == HEADS_PER_CORE - 1))
                            osb = tpool.tile([128, 512], F32, name="osb", tag="osb",
                                             bufs=3)
                            if qb == NB - 1:
                                nc.scalar.copy(osb[:], po[:])
                            else:
                                nc.vector.tensor_copy(osb[:], po[:])
                            nc.sync.dma_start(
                                out_d.ap()[128 * stt:128 * (stt + 1),
                                           512 * ob:512 * (ob + 1)],
                                osb[:])

    nc.compile()
    return nc


def _get_nc():
    if "nc" not in _CACHE:
        _CACHE["nc"] = _build()
    return _CACHE["nc"]


def _make_in_maps(x, W_query, W_DKV, W_UK, W_UV, W_out):
    mask = np.triu(np.ones((128, 128), dtype=np.float16))
    wdkv16 = W_DKV.astype(np.float16)
    xT16 = [x[b].T.astype(np.float16) for b in range(B)]
    in_maps = []
    for c in range(NCORES):
        b = c // 4
        g = c % 4
        cols = slice(512 * g, 512 * (g + 1))
        in_maps.append({
            "xt": xT16[b],
            "wq": W_query[:, cols].astype(np.float16),
            "wdkv": wdkv16,
            "wuk": W_UK[:, cols].astype(np.float16),
            "wuv": W_UV[:, cols].astype(np.float16),
            "wout": W_out[cols, :].astype(np.float16),
            "mask": mask,
        })
    return in_maps


def run_on_device(x, W_query, W_DKV, W_UK, W_UV, W_out, **run_kwargs):
    nc = _get_nc()
    in_maps = _make_in_maps(x, W_query, W_DKV, W_UK, W_UV, W_out)
    return bass_utils.run_bass_kernel_spmd(
        nc, in_maps, core_ids=list(range(NCORES)), **run_kwargs)


def kernel(x, W_query, W_DKV, W_UK, W_UV, W_out, b_out):
    x = np.asarray(x, dtype=np.float32)
    W_query = np.asarray(W_query, dtype=np.float32)
    W_DKV = np.asarray(W_DKV, dtype=np.float32)
    W_UK = np.asarray(W_UK, dtype=np.float32)
    W_UV = np.asarray(W_UV, dtype=np.float32)
    W_out = np.asarray(W_out, dtype=np.float32)
    b_out = np.asarray(b_out, dtype=np.float32)

    res = None
    for attempt in range(3):
        try:
            res = run_on_device(x, W_query, W_DKV, W_UK, W_UV, W_out)
            break
        except Exception:
            if attempt == 2:
                raise
    out = np.empty((B, S, DOUT), dtype=np.float32)
    for b in range(B):
        acc = res.results[4 * b]["out"].copy()
        for g in range(1, 4):
            acc += res.results[4 * b + g]["out"]
        out[b] = acc + b_out[None, :]
    return out


# revision 36
# speedup vs baseline: 21023.7002x; 1.0013x over previous
"""MultiHeadLatentAttention TRN2 kernel.

Sharding: 8 cores = 2 batches x 4 head-groups (4 heads of 128 dims each).
Each core computes, for its (batch, 4 heads):
    qT_h = Wq_h^T xT          [hd, S]     (fp16 matmuls, fp32 psum)
    latT = Wdkv^T xT          [256, S]
    kT_h = Wuk_h^T latT       [hd, S]
    v_h  = latT^T Wuv_h       [S, hd]
    scoresT = k qT            [keys, q]   (transposed scores: no transposes needed)
    expT = exp(scale*scoresT) (causal: skip invalid blocks, tri-mask diagonal)
    den  = ones^T expT        [128, q]    (all-ones stationary matmul = sum over
                                           keys AND broadcast across partitions)
    ctxT = v^T expT / den     [hd, q]
    part = sum_h ctxT_h^T Wout_h  [S, dout]  (row-parallel out-proj partial)
Host sums the 4 partials per batch and adds b_out.
"""

import sys

_BASS_REPO = "/opt/trn_rl_repo"
if _BASS_REPO not in sys.path:
    sys.path.insert(0, _BASS_REPO)

import numpy as np

import concourse.bass as bass  # noqa: F401
import concourse.mybir as mybir
import concourse.tile as tile
from concourse import bacc, bass_utils

F32 = mybir.dt.float32
F16 = mybir.dt.float16

B = 2
S = 2048
DIN = 2048
DOUT = 2048
NH = 16
HD = 128
LAT = 256
NCORES = 8
HEADS_PER_CORE = 4
COLS_PER_CORE = HEADS_PER_CORE * HD  # 512

KC = DIN // 128  # 16 contraction chunks over d_in
NB = S // 512    # 4 blocks of 512 over S
NT = S // 128    # 16 tiles of 128 over S
SCALE = 1.0 / float(np.sqrt(HD))

_CACHE = {}


def _build():
    nc = bacc.Bacc("TRN2", target_bir_lowering=False, debug=False,
                   num_devices=NCORES)

    xt_d = nc.dram_tensor("xt", [DIN, S], F16, kind="ExternalInput")
    wq_d = nc.dram_tensor("wq", [DIN, COLS_PER_CORE], F16, kind="ExternalInput")
    wdkv_d = nc.dram_tensor("wdkv", [DIN, LAT], F16, kind="ExternalInput")
    wuk_d = nc.dram_tensor("wuk", [LAT, COLS_PER_CORE], F16, kind="ExternalInput")
    wuv_d = nc.dram_tensor("wuv", [LAT, COLS_PER_CORE], F16, kind="ExternalInput")
    wout_d = nc.dram_tensor("wout", [COLS_PER_CORE, DOUT], F16, kind="ExternalInput")
    mask_d = nc.dram_tensor("mask", [128, 128], F16, kind="ExternalInput")
    out_d = nc.dram_tensor("out", [S, DOUT], F32, kind="ExternalOutput")

    Exp = mybir.ActivationFunctionType.Exp

    with tile.TileContext(nc) as tc:
        with (
            tc.tile_pool(name="consts", bufs=1) as cpool,
            tc.tile_pool(name="wts", bufs=1) as wpool,
            tc.tile_pool(name="acts", bufs=1) as apool,
            tc.tile_pool(name="temps", bufs=1) as tpool,
        ):
            # ---- constants ----
            ones_t = cpool.tile([128, 512], F16, name="ones_t", tag="ones_t")
            nc.vector.memset(ones_t[:], 1.0)
            neg_t = cpool.tile([128, 128], F16, name="neg_t", tag="neg_t")
            nc.vector.memset(neg_t[:], -30000.0)
            mask_t = cpool.tile([128, 128], F16, name="mask_t", tag="mask_t")
            nc.scalar.dma_start(mask_t[:], mask_d.ap())

            # ---- weights ----
            # xt/wdkv stream on the sync HWDGE ring (feed the first matmuls);
            # everything else loads in parallel on the scalar ring.
            wq = []
            wdkv = []
            xt = []
            for k in range(KC):
                t = wpool.tile([128, S], F16, name=f"xt{k}", tag=f"xt{k}")
                eng = nc.sync if k % 2 == 0 else nc.scalar
                eng.dma_start(t[:], xt_d.ap()[128 * k:128 * (k + 1), :])
                xt.append(t)
                t = wpool.tile([128, LAT], F16, name=f"wdkv{k}", tag=f"wdkv{k}")
                nc.sync.dma_start(t[:], wdkv_d.ap()[128 * k:128 * (k + 1), :])
                wdkv.append(t)
                t = wpool.tile([128, COLS_PER_CORE], F16, name=f"wq{k}", tag=f"wq{k}")
                nc.scalar.dma_start(t[:], wq_d.ap()[128 * k:128 * (k + 1), :])
                wq.append(t)
            wuk = []
            wuv = []
            for m in range(LAT // 128):
                t = wpool.tile([128, COLS_PER_CORE], F16, name=f"wuk{m}", tag=f"wuk{m}")
                nc.scalar.dma_start(t[:], wuk_d.ap()[128 * m:128 * (m + 1), :])
                wuk.append(t)
                t = wpool.tile([128, COLS_PER_CORE], F16, name=f"wuv{m}", tag=f"wuv{m}")
                nc.scalar.dma_start(t[:], wuv_d.ap()[128 * m:128 * (m + 1), :])
                wuv.append(t)
            wout = []
            for h in range(HEADS_PER_CORE):
                t = wpool.tile([128, DOUT], F16, name=f"wout{h}", tag=f"wout{h}")
                nc.scalar.dma_start(t[:], wout_d.ap()[128 * h:128 * (h + 1), :])
                wout.append(t)

            # ---- persistent activations ----
            latT = [apool.tile([128, S], F16, name=f"latT{m}", tag=f"latT{m}")
                    for m in range(LAT // 128)]
            qT = [apool.tile([128, S], F16, name=f"qT{h}", tag=f"qT{h}")
                  for h in range(HEADS_PER_CORE)]
            kT = [apool.tile([128, S], F16, name=f"kT{h}", tag=f"kT{h}")
                  for h in range(HEADS_PER_CORE)]
            vt = [apool.tile([128, S], F16, name=f"vt{h}", tag=f"vt{h}")
                  for h in range(HEADS_PER_CORE)]
            ctxT = [apool.tile([128, S], F16, name=f"ctxT{h}", tag=f"ctxT{h}")
                    for h in range(HEADS_PER_CORE)]

            # ================= phase 1: projections =================
            with tc.tile_pool(name="pproj", bufs=8, space="PSUM") as pproj:
                # PE warmup: HAM-warm the array while input DMAs stream in.
                warm = pproj.tile([128, 512], F32, name="warm", tag="pp")
                for _ in range(40):
                    nc.tensor.matmul(warm[:, 0:128], ones_t[:, 0:128],
                                     ones_t[:, 0:128], start=True, stop=True)

                def kmajor(groups, lhs_of, rhs_of, nk, out_of, copy_eng="scalar"):
                    """Accumulate len(groups) psum banks over nk chunks,
                    chunk-major so compute starts on the first DMA."""
                    pls = [pproj.tile([128, 512], F32, name=f"pp{i}", tag="pp")
                           for i in range(len(groups))]
                    for k in range(nk):
                        for i, g in enumerate(groups):
                            nc.tensor.matmul(pls[i][:], lhs_of(k, g), rhs_of(k, g),
                                             start=(k == 0), stop=(k == nk - 1))
                    for i, g in enumerate(groups):
                        if copy_eng == "scalar":
                            nc.scalar.copy(out_of(g), pls[i][:])
                        else:
                            nc.vector.tensor_copy(out_of(g), pls[i][:])

                # latT = Wdkv^T xT   (8 groups: 2 m x 4 sb)
                kmajor(
                    [(m, sb) for m in range(2) for sb in range(NB)],
                    lambda k, g: wdkv[k][:, 128 * g[0]:128 * (g[0] + 1)],
                    lambda k, g: xt[k][:, 512 * g[1]:512 * (g[1] + 1)],
                    KC,
                    lambda g: latT[g[0]][:, 512 * g[1]:512 * (g[1] + 1)])

                # qT_h = Wq_h^T xT   (two batches of 8 groups: 2 h x 4 sb)
                for h0 in (0, 2):
                    kmajor(
                        [(h0 + dh, sb) for dh in range(2) for sb in range(NB)],
                        lambda k, g: wq[k][:, 128 * g[0]:128 * (g[0] + 1)],
                        lambda k, g: xt[k][:, 512 * g[1]:512 * (g[1] + 1)],
                        KC,
                        lambda g: qT[g[0]][:, 512 * g[1]:512 * (g[1] + 1)])

                # kT_h = Wuk_h^T latT
                kmajor(
                    [(h, sb) for h in range(2) for sb in range(NB)],
                    lambda k, g: wuk[k][:, 128 * g[0]:128 * (g[0] + 1)],
                    lambda k, g: latT[k][:, 512 * g[1]:512 * (g[1] + 1)],
                    2,
                    lambda g: kT[g[0]][:, 512 * g[1]:512 * (g[1] + 1)],
                    copy_eng="vector")
                kmajor(
                    [(h, sb) for h in (2, 3) for sb in range(NB)],
                    lambda k, g: wuk[k][:, 128 * g[0]:128 * (g[0] + 1)],
                    lambda k, g: latT[k][:, 512 * g[1]:512 * (g[1] + 1)],
                    2,
                    lambda g: kT[g[0]][:, 512 * g[1]:512 * (g[1] + 1)],
                    copy_eng="vector")

                # v_h = latT^T Wuv_h, stored [128, st*128+d]
                for h in range(HEADS_PER_CORE):
                    hs = slice(128 * h, 128 * (h + 1))
                    for st4 in range(NB):
                        pv = pproj.tile([128, 512], F32, name="pv", tag="pp")
                        for j in range(4):
                            stt = 4 * st4 + j
                            for m in range(LAT // 128):
                                nc.tensor.matmul(
                                    pv[:, 128 * j:128 * (j + 1)],
                                    latT[m][:, 128 * stt:128 * (stt + 1)],
                                    wuv[m][:, hs],
                                    start=(m == 0), stop=(m == LAT // 128 - 1))
                        nc.vector.tensor_copy(vt[h][:, 512 * st4:512 * (st4 + 1)],
                                              pv[:])


            # ========= phase 2: attention + interleaved out-proj =========
            # key tiles processed in pairs -> one [128,1024] exp per pair.
            # qb-outer / h-inner so each q-block's out-projection (PE-heavy,
            # ACT-idle) overlaps the next block's ACT-paced attention.
            with (
                tc.tile_pool(name="psc", bufs=2, space="PSUM") as psc,
                tc.tile_pool(name="pctx", bufs=2, space="PSUM") as pctx,
                tc.tile_pool(name="pden", bufs=2, space="PSUM") as pden,
            ):
                for qb in range(NB):
                    for h in range(HEADS_PER_CORE):
                        ps_ctx = pctx.tile([128, 512], F32, name="ps_ctx", tag="ctx")
                        ps_den = pden.tile([128, 512], F32, name="ps_den", tag="den")
                        nkt = 4 * qb + 4
                        for kt0 in range(0, nkt, 2):
                            pair = (kt0, kt0 + 1)
                            # valid q start (block-local) per kt; pair shares
                            # the wider (earlier) start col0 of ktA
                            djA = pair[0] - 4 * qb
                            col0 = 128 * djA if djA >= 0 else 0
                            qhi = 512 * (qb + 1)
                            ps_sc = psc.tile([128, 1024], F32, name="ps_sc",
                                             tag="sc")
                            ex = tpool.tile([128, 1024], F16, name="ex", tag="ex",
                                            bufs=3)
                            for half, kt in enumerate(pair):
                                dj = kt - 4 * qb
                                c = 128 * dj if dj >= 0 else 0
                                if c > 0:
                                    # fill the invalid strip with -huge so the
                                    # wide exp below lands exact zeros there
                                    nc.tensor.matmul(
                                        ps_sc[:, 512 * half:512 * half + c],
                                        neg_t[:], ones_t[:, 0:c],
                                        start=True, stop=True,
                                        skip_group_check=True)
                                nc.tensor.matmul(
                                    ps_sc[:, 512 * half + c:512 * (half + 1)],
                                    kT[h][:, 128 * kt:128 * (kt + 1)],
                                    qT[h][:, 512 * qb + c:qhi],
                                    start=True, stop=True,
                                    skip_group_check=True)
                            # one wide exp for the pair (psum -> sbuf fp16)
                            nc.scalar.activation(ex[:, col0:1024],
                                                 ps_sc[:, col0:1024], Exp,
                                                 scale=SCALE)
                            for half, kt in enumerate(pair):
                                dj = kt - 4 * qb
                                if dj >= 0:
                                    c = 128 * dj
                                    nc.vector.tensor_mul(
                                        ex[:, 512 * half + c:512 * half + c + 128],
                                        ex[:, 512 * half + c:512 * half + c + 128],
                                        mask_t[:])
                            # pair-sum on DVE halves the denominator matmuls
                            exs = tpool.tile([128, 512], F16, name="exs",
                                             tag="exs", bufs=3)
                            nc.vector.tensor_add(exs[:, col0:512],
                                                 ex[:, col0:512],
                                                 ex[:, 512 + col0:1024])
                            nc.tensor.matmul(
                                ps_den[:, col0:512],
                                ones_t[:, 0:128],
                                exs[:, col0:512],
                                start=(kt0 == 0), stop=(kt0 == nkt - 2))
                            for half, kt in enumerate(pair):
                                nc.tensor.matmul(
                                    ps_ctx[:, col0:512],
                                    vt[h][:, 128 * kt:128 * (kt + 1)],
                                    ex[:, 512 * half + col0:512 * (half + 1)],
                                    start=(kt0 == 0 and half == 0),
                                    stop=(kt == nkt - 1))
                        rden = tpool.tile([128, 512], F32, name="rden", tag="rden",
                                          bufs=2)
                        nc.vector.reciprocal_approx_fast(rden[:], ps_den[:])
                        nc.vector.tensor_mul(ctxT[h][:, 512 * qb:512 * (qb + 1)],
                                             ps_ctx[:], rden[:])

                    # out-projection for this q-block's 4 S-tiles
                    # (psum slots shared with the den tag)
                    for stt in range(4 * qb, 4 * qb + 4):
                        for ob in range(NB):
                            po = pden.tile([128, 512], F32, name="po", tag="den")
                            for h in range(HEADS_PER_CORE):
                                nc.tensor.matmul(
                                    po[:],
                                    ctxT[h][:, 128 * stt:128 * (stt + 1)],
                                    wout[h][:, 512 * ob:512 * (ob + 1)],
                                    start=(h == 0), stop=(h == HEADS_PER_CORE - 1))
                            osb = tpool.tile([128, 512], F32, name="osb", tag="osb",
                                             bufs=3)
                            if qb == NB - 1:
                                nc.scalar.copy(osb[:], po[:])
                            else:
                                nc.vector.tensor_copy(osb[:], po[:])
                            nc.sync.dma_start(
                                out_d.ap()[128 * stt:128 * (stt + 1),
                                           512 * ob:512 * (ob + 1)],
                                osb[:])

    nc.compile()
    return nc


def _get_nc():
    if "nc" not in _CACHE:
        _CACHE["nc"] = _build()
    return _CACHE["nc"]


def _make_in_maps(x, W_query, W_DKV, W_UK, W_UV, W_out):
    mask = np.triu(np.ones((128, 128), dtype=np.float16))
    wdkv16 = W_DKV.astype(np.float16)
    xT16 = [x[b].T.astype(np.float16) for b in range(B)]
    in_maps = []
    for c in range(NCORES):
        b = c // 4
        g = c % 4
        cols = slice(512 * g, 512 * (g + 1))
        in_maps.append({
            "xt": xT16[b],
            "wq": W_query[:, cols].astype(np.float16),
            "wdkv": wdkv16,
            "wuk": W_UK[:, cols].astype(np.float16),
            "wuv": W_UV[:, cols].astype(np.float16),
            "wout": W_out[cols, :].astype(np.float16),
            "mask": mask,
        })
    return in_maps


def run_on_device(x, W_query, W_DKV, W_UK, W_UV, W_out, **run_kwargs):
    nc = _get_nc()
    in_maps = _make_in_maps(x, W_query, W_DKV, W_UK, W_UV, W_out)
    return bass_utils.run_bass_kernel_spmd(
        nc, in_maps, core_ids=list(range(NCORES)), **run_kwargs)


def kernel(x, W_query, W_DKV, W_UK, W_UV, W_out, b_out):
    x = np.asarray(x, dtype=np.float32)
    W_query = np.asarray(W_query, dtype=np.float32)
    W_DKV = np.asarray(W_DKV, dtype=np.float32)
    W_UK = np.asarray(W_UK, dtype=np.float32)
    W_UV = np.asarray(W_UV, dtype=np.float32)
    W_out = np.asarray(W_out, dtype=np.float32)
    b_out = np.asarray(b_out, dtype=np.float32)

    res = None
    for attempt in range(3):
        try:
            res = run_on_device(x, W_query, W_DKV, W_UK, W_UV, W_out)
            break
        except Exception:
            if attempt == 2:
                raise
    out = np.empty((B, S, DOUT), dtype=np.float32)
    for b in range(B):
        acc = res.results[4 * b]["out"].copy()
        for g in range(1, 4):
            acc += res.results[4 * b + g]["out"]
        out[b] = acc + b_out[None, :]
    return out
